# revision 1
# baseline (speedup 1.0000x reference)
"""BitPackedLinear Trainium2 kernel (8-core SPMD, token-sharded).

y = x @ W.T + bias, W = unpack_bits(packed_weight) in {-1,+1}, shapes:
  x [2, 2048, 4096] f32, packed_weight [4096, 512] u8, bias [4096] f32.

Sharding: data-parallel over tokens (4096 tokens -> 512/core). Each core
computes y_c = x_c @ W.T + bias for its token shard against the full
weight; the host just concatenates shards.

Device algorithm per core (all matmuls bf16 at 1 cyc/row):
  - Contraction (i) is tiled bit-sliced: i-tile (kt, b) = {8*(128*kt+k')+b},
    which makes every W^T [i,o] tile a single shift+mask op from transposed
    packed bytes (no cross-partition fanout).
  - byteT[k',kt,o] = pw[o,128*kt+k'] via PE pass-through transposes,
    built just-in-time per o-slab.
  - Unpack writes the bf16 BIT PATTERN of {0, 2.0} with pure bitvec ops:
    (u16(byte) << (14-b)) & 0x4000, then the tile is bitcast to bf16.
    (walrus forbids dtype casts on bitvec tensor_scalar ops.)
  - x is cast f32->bf16 in-flight by SWDGE DMA; xT tiles via PE transpose
    (bf16, 1 cyc/row) in i-tile order so the matmul stream chases them.
  - main matmuls: psum[t,o] += xT_it.T @ W2_it  (= 2*(x@B.T), y-natural)
  - bias via a rank-1 f32r matmul (K=1): psum += ones.T @ bias_row.
  - rowsum correction: s_col[t] = sum_i x_bf[t,i] via DVE reduces of the
    bf16 x chunks (exactly consistent with the bf16 matmul inputs);
    epilogue fuses y = psum - s_col into the PSUM->SBUF copy.
"""
import sys

sys.path.insert(0, "/opt/trn_rl_repo")
from contextlib import ExitStack

import numpy as np

import concourse.tile as tile
from concourse import bacc, mybir
from concourse.bass import ts
from concourse.bass_utils import run_bass_kernel_spmd
from concourse.masks import make_identity

F32 = mybir.dt.float32
F32R = mybir.dt.float32r
BF16 = mybir.dt.bfloat16
U8 = mybir.dt.uint8
U16 = mybir.dt.uint16
P = 128

N_CORES = 8
B_DIM, S_DIM, I_DIM, O_DIM = 2, 2048, 4096, 4096
T_FULL = B_DIM * S_DIM          # 4096 tokens
T_SHARD = T_FULL // N_CORES     # 512 tokens per core
OUT_NAME = "y"
OUT_SHAPE = (T_SHARD, O_DIM)


def build(T=T_SHARD, I=I_DIM, O=O_DIM, O_SLAB=512, n_cores=N_CORES, n_reps=1,
          byte_mode="pe"):
    assert I % 1024 == 0 and T % P == 0 and O % P == 0 and O % O_SLAB == 0
    KT = I // 1024          # 128-byte groups along i
    IT = KT * 8             # bit-sliced i-tiles
    TT = T // P             # token tiles
    K = I // 8              # packed bytes per weight row
    NSLAB = O // O_SLAB
    OSL_T = O_SLAB // P
    SHB, MASK = 14, 0x4000  # u16 bf16-pattern unpack constants

    nc = bacc.Bacc("TRN2", target_bir_lowering=False, debug=False,
                   num_devices=n_cores)
    x_d = nc.dram_tensor("x", [T, I], F32, kind="ExternalInput").ap()
    pw_d = nc.dram_tensor("pw", [O, K], U8, kind="ExternalInput").ap()
    bias_d = nc.dram_tensor("bias", [O], F32, kind="ExternalInput").ap()
    y_d = nc.dram_tensor(OUT_NAME, [T, O], F32, kind="ExternalOutput").ap()

    with tile.TileContext(nc) as tc:
        with ExitStack() as ctx:
            const = ctx.enter_context(tc.tile_pool(name="const", bufs=1))
            persist = ctx.enter_context(tc.tile_pool(name="persist", bufs=1))

            ident_bf = const.tile([P, P], BF16)
            make_identity(nc, ident_bf[:])
            ones_r = const.tile([1, P], F32R)
            bias_r = const.tile([1, O], F32R)
            stage = ctx.enter_context(tc.tile_pool(name="stage", bufs=1))

            def emit_bias_stage():
                ones_f32 = stage.tile([1, P], F32)
                nc.vector.memset(ones_f32[:], 1.0)
                nc.vector.tensor_copy(out=ones_r[:], in_=ones_f32[:])
                bias_f32 = stage.tile([1, O], F32)
                nc.sync.dma_start(
                    bias_f32[:], bias_d.rearrange("(b o) -> b o", b=1)
                )
                nc.vector.tensor_copy(out=bias_r[:], in_=bias_f32[:])

            byteT = persist.tile([P, KT, O], U16)
            xT = persist.tile([P, IT, T], BF16)
            pw_ap = pw_d.rearrange("(ot p) k -> p ot k", p=P)
            pw16_d = nc.dram_tensor("pw16", [O, K], U16).ap()

            pk_pool = ctx.enter_context(tc.tile_pool(name="pk", bufs=2))
            pkbf_pool = ctx.enter_context(tc.tile_pool(name="pkbf", bufs=2))
            ps_tr = ctx.enter_context(
                tc.tile_pool(name="ps_tr", bufs=3, space="PSUM")
            )
            xnat_pool = ctx.enter_context(
                tc.tile_pool(name="xnat", bufs=max(2 * TT, KT * TT - 4))
            )
            x32_pool = ctx.enter_context(tc.tile_pool(name="x32", bufs=2))
            scol_pool = ctx.enter_context(tc.tile_pool(name="scol", bufs=2))
            wt_pool = ctx.enter_context(tc.tile_pool(name="wt", bufs=2))
            y_pool = ctx.enter_context(tc.tile_pool(name="ysb", bufs=3))
            ps_mm = ctx.enter_context(
                tc.tile_pool(name="ps_mm", bufs=4, space="PSUM")
            )
            ps_b_pool = ctx.enter_context(
                tc.tile_pool(name="ps_b", bufs=1, space="PSUM")
            )

            def byte_slab(sl):
                """Fill byteT[:, :, sl*O_SLAB:(sl+1)*O_SLAB] from pw (JIT)."""
                pk = pk_pool.tile([P, OSL_T, K], U8)
                nc.sync.dma_start(pk[:], pw_ap[:, ts(sl, OSL_T), :])
                for otl in range(OSL_T):
                    ot = sl * OSL_T + otl
                    pkbf = pkbf_pool.tile([P, K], BF16)
                    nc.any.tensor_copy(out=pkbf[:], in_=pk[:, otl, :])
                    for kt in range(KT):
                        ps = ps_tr.tile([P, P], BF16, tag="tr_ps")
                        nc.tensor.transpose(ps[:], pkbf[:, ts(kt, P)], ident_bf[:])
                        nc.any.tensor_copy(out=byteT[:, kt, ts(ot, P)], in_=ps[:])

            for _rep in range(n_reps):
                if byte_mode == "dmat":
                    # bytes: u8->u16 cast bounce through DRAM, then one
                    # XBAR transpose-DMA per kt fills byteT[:, kt, :] whole
                    nc.gpsimd.dma_start(out=pw16_d[:], in_=pw_d[:])
                    for kt in range(KT):
                        nc.sync.dma_start_transpose(
                            byteT[:, kt, :], pw16_d[:, ts(kt, P)]
                        )
                else:
                    # slab 0 first: DVE/ACT work exists while x DMAs land
                    # (hybrid mode: slabs 1+ come via XBAR, emitted later)
                    byte_slab(0)

                # x chunks, kt-major. kt=0 goes via fast HWDGE as f32 +
                # a DVE cast (DVE is idle this early; SWDGE descriptor-gen
                # on the Q7 makes the first cast-DMA land ~13us in, which
                # stalls the PE). kt>=1 use SWDGE cast-DMA f32->bf16.
                xns = {}
                for kt in range(KT):
                    for tt in range(TT):
                        src_ap = x_d[ts(tt, P), ts(kt, 1024)].rearrange(
                            "p (k b) -> p k b", b=8
                        )
                        xn = xnat_pool.tile([P, P, 8], BF16, tag="xn16")
                        if kt == 0:
                            x32 = x32_pool.tile([P, P, 8], F32)
                            nc.sync.dma_start(x32[:], src_ap)
                            nc.vector.tensor_copy(out=xn[:], in_=x32[:])
                        else:
                            nc.gpsimd.dma_start(xn[:], src_ap)
                        xns[kt, tt] = xn

                if byte_mode == "hybrid":
                    # slabs 1..7 bytes via XBAR transpose-DMAs; cast-DMA is
                    # emitted after the x chunks so it loses the Q7 race.
                    # Each dest is one contiguous 1KB run per partition.
                    nc.gpsimd.dma_start(
                        out=pw16_d[O_SLAB:, :], in_=pw_d[O_SLAB:, :]
                    )
                    for sl in range(1, NSLAB):
                        for kt in range(KT):
                            nc.sync.dma_start_transpose(
                                byteT[:, kt, ts(sl, O_SLAB)],
                                pw16_d[ts(sl, O_SLAB), ts(kt, P)],
                            )

                # xT via PE transposes, i-tile-major so matmuls can chase
                for kt in range(KT):
                    for b in range(8):
                        it = kt * 8 + b
                        for tt in range(TT):
                            ps = ps_tr.tile([P, P], BF16, tag="tr_ps")
                            nc.tensor.transpose(
                                ps[:], xns[kt, tt][:, :, b], ident_bf[:]
                            )
                            nc.any.tensor_copy(out=xT[:, it, ts(tt, P)], in_=ps[:])

                if _rep == 0:
                    emit_bias_stage()

                # s_col[t] = sum_i x_bf[t, i] on DVE (consistent with MM
                # input); emitted after slab 0's unpack so it doesn't delay
                # the first matmuls
                s_col = scol_pool.tile([P, TT], F32)
                parts = scol_pool.tile([P, TT, KT], F32, tag="sparts")

                def emit_s_col():
                    for tt in range(TT):
                        for kt in range(KT):
                            nc.vector.tensor_reduce(
                                out=parts[:, tt, kt:kt + 1],
                                in_=xns[kt, tt][:],
                                op=mybir.AluOpType.add,
                                axis=mybir.AxisListType.XY,
                            )
                        nc.vector.tensor_reduce(
                            out=s_col[:, tt:tt + 1],
                            in_=parts[:, tt, :],
                            op=mybir.AluOpType.add,
                            axis=mybir.AxisListType.X,
                        )

                # main o-slab loop
                for sl in range(NSLAB):
                    if sl > 0 and byte_mode == "pe":
                        byte_slab(sl)
                    wt = wt_pool.tile([P, IT, O_SLAB], U16)
                    for kt in range(KT):
                        for b in range(8):
                            it = kt * 8 + b
                            nc.vector.tensor_scalar(
                                out=wt[:, it, :],
                                in0=byteT[:, kt, ts(sl, O_SLAB)],
                                scalar1=SHB - b, scalar2=MASK,
                                op0=mybir.AluOpType.logical_shift_left,
                                op1=mybir.AluOpType.bitwise_and,
                            )
                    if sl == 0:
                        emit_s_col()
                    ps_bias = ps_b_pool.tile([P, O_SLAB], F32)
                    nc.tensor.matmul(
                        ps_bias[:], ones_r[:], bias_r[:, ts(sl, O_SLAB)],
                        start=True, stop=True,
                    )
                    bbc = y_pool.tile([P, O_SLAB], F32, name="bbc", tag="y_sb")
                    nc.any.tensor_copy(out=bbc[:], in_=ps_bias[:])
                    for tsub in range(TT):
                        ps = ps_mm.tile([P, O_SLAB], F32)
                        for it in range(IT):
                            nc.tensor.matmul(
                                ps[:],
                                xT[:, it, ts(tsub, P)],
                                wt[:, it, :].bitcast(BF16),
                                start=(it == 0), stop=(it == IT - 1),
                            )
                        y_sb = y_pool.tile([P, O_SLAB], F32)
                        nc.vector.scalar_tensor_tensor(
                            out=y_sb[:], in0=ps[:],
                            scalar=s_col[:, tsub:tsub + 1],
                            in1=bbc[:],
                            op0=mybir.AluOpType.subtract,
                            op1=mybir.AluOpType.add,
                        )
                        nc.sync.dma_start(
                            y_d[ts(tsub, P), ts(sl, O_SLAB)], y_sb[:]
                        )

    nc.compile()
    return nc


_NC = None


def _get_nc():
    global _NC
    if _NC is None:
        _NC = build()
    return _NC


def run(x, packed_weight, bias, trace=False):
    x = np.ascontiguousarray(np.asarray(x, dtype=np.float32))
    pw = np.ascontiguousarray(np.asarray(packed_weight, dtype=np.uint8))
    bias = np.ascontiguousarray(np.asarray(bias, dtype=np.float32))
    assert x.shape == (B_DIM, S_DIM, I_DIM)
    assert pw.shape == (O_DIM, I_DIM // 8)
    assert bias.shape == (O_DIM,)

    nc = _get_nc()
    xs = x.reshape(T_FULL, I_DIM)
    in_maps = [
        {
            "x": np.ascontiguousarray(xs[c * T_SHARD:(c + 1) * T_SHARD]),
            "pw": pw,
            "bias": bias,
        }
        for c in range(N_CORES)
    ]
    res = run_bass_kernel_spmd(nc, in_maps, list(range(N_CORES)), trace=trace)
    y = np.concatenate(
        [res.results[c][OUT_NAME] for c in range(N_CORES)], axis=0
    )
    return y.reshape(B_DIM, S_DIM, O_DIM), res


def kernel(x, packed_weight, bias):
    y, _ = run(x, packed_weight, bias, trace=False)
    return y



# revision 7
# speedup vs baseline: 1.4426x; 1.4426x over previous
"""BitPackedLinear Trainium2 kernel (8-core SPMD, token-sharded, fp8 DoubleRow).

y = x @ W.T + bias, W = unpack_bits(packed_weight) in {-1,+1}, shapes:
  x [2, 2048, 4096] f32, packed_weight [4096, 512] u8, bias [4096] f32.

Sharding: data-parallel over tokens (4096 tokens -> 512/core). Each core
computes y_c = x_c @ W.T + bias for its token shard against the full
weight; the host just concatenates shards.

Device algorithm per core (matmuls are fp8e4 DoubleRow at 0.5 cyc/row,
one instruction contracts TWO 128-deep k-tiles):
  - x is split as x_bf = hi + lo with hi = e4m3(x_bf), lo = x_bf - hi
    (lo is exactly representable in e4m3, so hi+lo == bf16(x) exactly;
    end-to-end rel err ~1.7e-3, dominated by the bf16 load cast).
  - i-tiling: i = 2048h + 16j + 8p + b with j the PE partition, (p, b)
    byte-parity/bit, h the 2048-halves. The DoubleRow k-tile pair is p.
  - weights: pw is viewed as u16 [4096, 256] (byte pairs along k) and
    XBAR-transpose-DMA'd to byteT2 [128j, h, o]; each u16 holds bytes
    (2j, 2j+1) of one o. A single u16 shift+mask (<<(6-b), & 0x4040)
    then yields the e4m3 BIT PATTERN of {0, 2.0} (0x40) for BOTH
    parities of bit b at once -> one DVE op per (h, b, o-slab), all in
    the DVE 4x perf mode. The u16 result bitcast to fp8 is the moving
    [j, p, o] operand.
  - x chunks arrive as f32->bf16 SWDGE cast-DMAs [128t, 2048i]; PE
    transposes (1 cyc/row) produce [j, t] bf16 tiles grouped in wide
    PSUM tiles; ACT casts psum->fp8 (hi), DVE subtracts psum - hi -> lo
    (mixed-dtype in, fp8 out).
  - psum[t, o] accumulates 32 DoubleRow matmuls (2 passes x 2 h x 8 b).
  - bias is pre-broadcast via a rank-1 f32r matmul into bbc; rowsum
    s[t] = sum_i bf16(x)[t, i] on Pool (consistent with hi+lo);
    epilogue fuses y = psum - s + bbc on DVE/Pool.
"""
import sys

sys.path.insert(0, "/opt/trn_rl_repo")
from contextlib import ExitStack

import numpy as np

import concourse.tile as tile
from concourse import bacc, mybir
from concourse.bass import ts
from concourse.bass_utils import run_bass_kernel_spmd
from concourse.masks import make_identity

F32 = mybir.dt.float32
F32R = mybir.dt.float32r
BF16 = mybir.dt.bfloat16
U16 = mybir.dt.uint16
F8 = mybir.dt.float8e4
P = 128

N_CORES = 8
B_DIM, S_DIM, I_DIM, O_DIM = 2, 2048, 4096, 4096
T_FULL = B_DIM * S_DIM          # 4096 tokens
T_SHARD = T_FULL // N_CORES     # 512 tokens per core
OUT_NAME = "y"


def build(T=T_SHARD, I=I_DIM, O=O_DIM, n_cores=N_CORES):
    H = I // 2048               # 2048-wide i-halves (j spans 16*128)
    TT = T // P                 # token tiles
    NB = 8                      # bits per byte
    K2 = I // 16                # u16 byte-pairs per weight row
    OSL = 512                   # o-slab width
    NSL = O // OSL

    nc = bacc.Bacc("TRN2", target_bir_lowering=False, debug=False,
                   num_devices=n_cores)
    x_d = nc.dram_tensor("x", [T, I], F32, kind="ExternalInput").ap()
    pw16_d = nc.dram_tensor("pw16", [O, K2], U16, kind="ExternalInput").ap()
    bias_d = nc.dram_tensor("bias", [O], F32, kind="ExternalInput").ap()
    y_d = nc.dram_tensor(OUT_NAME, [T, O], F32, kind="ExternalOutput").ap()

    with tile.TileContext(nc) as tc:
        with ExitStack() as ctx:
            const = ctx.enter_context(tc.tile_pool(name="const", bufs=1))
            persist = ctx.enter_context(tc.tile_pool(name="persist", bufs=1))
            stage = ctx.enter_context(tc.tile_pool(name="stage", bufs=1))

            ident_bf = const.tile([P, P], BF16)
            make_identity(nc, ident_bf[:])
            ones_r = const.tile([1, P], F32R)
            bias_r = const.tile([1, O], F32R)

            byteT2 = persist.tile([P, H, O], U16)
            # xT planes: [j, h, bh, tt, (b' p t)] fp8, 16KB/partition each
            xT_hi = persist.tile([P, H, 2, TT, 1024], F8)
            xT_lo = persist.tile([P, H, 2, TT, 1024], F8)

            xn_pool = ctx.enter_context(tc.tile_pool(name="xn", bufs=2 * H * TT))
            wt_pool = ctx.enter_context(tc.tile_pool(name="wt", bufs=3))
            scol_pool = ctx.enter_context(tc.tile_pool(name="scol", bufs=1))
            bbc_pool = ctx.enter_context(tc.tile_pool(name="bbc", bufs=2))
            y_pool = ctx.enter_context(tc.tile_pool(name="ysb", bufs=4))
            ps_tr = ctx.enter_context(
                tc.tile_pool(name="ps_tr", bufs=3, space="PSUM"))
            ps_mm = ctx.enter_context(
                tc.tile_pool(name="ps_mm", bufs=3, space="PSUM"))
            ps_b_pool = ctx.enter_context(
                tc.tile_pool(name="ps_b", bufs=1, space="PSUM"))

            # --- weight bytes: XBAR transpose-DMAs, slab-major ---
            for sl in range(NSL):
                for h in range(H):
                    nc.sync.dma_start_transpose(
                        byteT2[:, h, ts(sl, OSL)],
                        pw16_d[ts(sl, OSL), ts(h, P)],
                    )

            # --- x chunks: SWDGE f32->bf16 cast DMAs, h-major ---
            xns = {}
            for h in range(H):
                for tt in range(TT):
                    xn = xn_pool.tile([P, P, 16], BF16, tag="xn")
                    nc.gpsimd.dma_start(
                        xn[:].rearrange("t j q -> t (j q)"),
                        x_d[ts(tt, P), ts(h, 2048)],
                    )
                    xns[h, tt] = xn

            # --- bias/ones staging ---
            ones_f32 = stage.tile([1, P], F32)
            nc.vector.memset(ones_f32[:], 1.0)
            nc.vector.tensor_copy(out=ones_r[:], in_=ones_f32[:])
            bias_f32 = stage.tile([1, O], F32)
            nc.sync.dma_start(
                bias_f32[:], bias_d.rearrange("(b o) -> b o", b=1))
            nc.vector.tensor_copy(out=bias_r[:], in_=bias_f32[:])

            # --- unpack weight slabs 0,1 up front (DVE, 4x mode) ---
            def unpack(sl, wt):
                for h in range(H):
                    for b in range(NB):
                        if b < 7:
                            nc.vector.tensor_scalar(
                                out=wt[:, h, b, :],
                                in0=byteT2[:, h, ts(sl, OSL)],
                                scalar1=6 - b, scalar2=0x4040,
                                op0=mybir.AluOpType.logical_shift_left,
                                op1=mybir.AluOpType.bitwise_and,
                            )
                        else:
                            nc.vector.tensor_scalar(
                                out=wt[:, h, b, :],
                                in0=byteT2[:, h, ts(sl, OSL)],
                                scalar1=1, scalar2=0x4040,
                                op0=mybir.AluOpType.logical_shift_right,
                                op1=mybir.AluOpType.bitwise_and,
                            )

            wts = {}
            for sl in range(min(2, NSL)):
                wts[sl] = wt_pool.tile([P, H, NB, OSL], U16, name="wt", tag="wt")
                unpack(sl, wts[sl])

            # --- transposes + hi/lo split, chunk-arrival order ---
            for h in range(H):
                for tt in range(TT):
                    for bh in range(2):
                        ps = ps_tr.tile([P, 1024], BF16, tag="tr")
                        for bp in range(4):
                            for p in range(2):
                                nc.tensor.transpose(
                                    ps[:, ts(bp * 2 + p, P)],
                                    xns[h, tt][:, :, 8 * p + 4 * bh + bp],
                                    ident_bf[:],
                                )
                        nc.scalar.copy(out=xT_hi[:, h, bh, tt, :], in_=ps[:])
                        nc.vector.tensor_tensor(
                            out=xT_lo[:, h, bh, tt, :], in0=ps[:],
                            in1=xT_hi[:, h, bh, tt, :],
                            op=mybir.AluOpType.subtract,
                        )

            # --- rowsum s[t] on Pool (consistent with hi+lo == bf16 x) ---
            parts = scol_pool.tile([P, TT, H], F32)
            s_col = scol_pool.tile([P, TT], F32)
            for h in range(H):
                for tt in range(TT):
                    nc.vector.tensor_reduce(
                        out=parts[:, tt, h:h + 1],
                        in_=xns[h, tt][:],
                        op=mybir.AluOpType.add, axis=mybir.AxisListType.XY,
                    )
            for tt in range(TT):
                nc.vector.tensor_reduce(
                    out=s_col[:, tt:tt + 1], in_=parts[:, tt, :],
                    op=mybir.AluOpType.add, axis=mybir.AxisListType.X,
                )

            # --- main loop over o-slabs ---
            for sl in range(NSL):
                wt = wts.pop(sl)
                ps_bias = ps_b_pool.tile([P, OSL], F32)
                nc.tensor.matmul(
                    ps_bias[:], ones_r[:], bias_r[:, ts(sl, OSL)],
                    start=True, stop=True,
                )
                bbc = bbc_pool.tile([P, OSL], F32, tag="bbc")
                nc.scalar.copy(out=bbc[:], in_=ps_bias[:])

                for tt in range(TT):
                    ps = ps_mm.tile([P, OSL], F32)
                    n = 0
                    for plane in (xT_hi, xT_lo):
                        for h in range(H):
                            for bh in range(2):
                                for bp in range(4):
                                    rhs = wt[:, h, 4 * bh + bp, :].bitcast(
                                        F8).rearrange("j (o p) -> j p o", p=2)
                                    nc.tensor.matmul(
                                        ps[:],
                                        plane[:, h, bh, tt, ts(bp, 256)]
                                        .rearrange("j (p t) -> j p t", p=2),
                                        rhs,
                                        start=(n == 0), stop=(n == 31),
                                        perf_mode=mybir.MatmulPerfMode.DoubleRow,
                                    )
                                    n += 1
                    y_sb = y_pool.tile([P, OSL], F32)
                    nc.vector.scalar_tensor_tensor(
                        out=y_sb[:], in0=ps[:], scalar=s_col[:, tt:tt + 1], in1=bbc[:],
                        op0=mybir.AluOpType.subtract,
                        op1=mybir.AluOpType.add,
                    )
                    nc.sync.dma_start(y_d[ts(tt, P), ts(sl, OSL)], y_sb[:])

                if sl + 2 < NSL:
                    wts[sl + 2] = wt_pool.tile([P, H, NB, OSL], U16, name="wt", tag="wt")
                    unpack(sl + 2, wts[sl + 2])

    nc.compile()
    return nc


_NC = None


def _get_nc():
    global _NC
    if _NC is None:
        _NC = build()
    return _NC


def run(x, packed_weight, bias, trace=False):
    x = np.ascontiguousarray(np.asarray(x, dtype=np.float32))
    pw = np.ascontiguousarray(np.asarray(packed_weight).astype(np.uint8))
    bias = np.ascontiguousarray(np.asarray(bias, dtype=np.float32))
    assert x.shape == (B_DIM, S_DIM, I_DIM)
    assert pw.shape == (O_DIM, I_DIM // 8)
    assert bias.shape == (O_DIM,)

    nc = _get_nc()
    xs = x.reshape(T_FULL, I_DIM)
    pw16 = pw.view(np.uint16)
    in_maps = [
        {
            "x": np.ascontiguousarray(xs[c * T_SHARD:(c + 1) * T_SHARD]),
            "pw16": pw16,
            "bias": bias,
        }
        for c in range(N_CORES)
    ]
    res = run_bass_kernel_spmd(nc, in_maps, list(range(N_CORES)), trace=trace)
    y = np.concatenate(
        [res.results[c][OUT_NAME] for c in range(N_CORES)], axis=0
    )
    return y.reshape(B_DIM, S_DIM, O_DIM), res


def kernel(x, packed_weight, bias):
    y, _ = run(x, packed_weight, bias, trace=False)
    return y


# revision 9
# speedup vs baseline: 1.6390x; 1.1361x over previous
"""BitPackedLinear Trainium2 kernel (8-core SPMD, token-sharded, fp8 DoubleRow).

y = x @ W.T + bias, W = unpack_bits(packed_weight) in {-1,+1}, shapes:
  x [2, 2048, 4096] f32, packed_weight [4096, 512] u8, bias [4096] f32.

Sharding: data-parallel over tokens (4096 tokens -> 512/core). Each core
computes y_c = x_c @ W.T + bias for its token shard against the full
weight; the host just concatenates shards.

Device algorithm per core (matmuls are fp8e4 DoubleRow at 0.5 cyc/row,
one instruction contracts TWO 128-deep k-tiles):
  - x is split as x_bf = hi + lo with hi = e4m3(x_bf), lo = x_bf - hi
    (lo is exactly representable in e4m3, so hi+lo == bf16(x) exactly;
    end-to-end rel err ~1.7e-3, dominated by the bf16 load cast).
  - i-tiling: i = 2048h + 16j + 8p + b with j the PE partition, (p, b)
    byte-parity/bit, h the 2048-halves. The DoubleRow k-tile pair is p.
  - weights: pw is viewed as u16 [4096, 256] (byte pairs along k) and
    XBAR-transpose-DMA'd to byteT2 [128j, h, o]; each u16 holds bytes
    (2j, 2j+1) of one o. A single u16 shift+mask (<<(6-b), & 0x4040)
    then yields the e4m3 BIT PATTERN of {0, 2.0} (0x40) for BOTH
    parities of bit b at once -> one DVE op per (h, b, o-slab), all in
    the DVE 4x perf mode. The u16 result bitcast to fp8 is the moving
    [j, p, o] operand.
  - x chunks arrive as f32->bf16 SWDGE cast-DMAs [128t, 2048i]; PE
    transposes (1 cyc/row) produce [j, t] bf16 tiles grouped in wide
    PSUM tiles; ACT casts psum->fp8 (hi), DVE subtracts psum - hi -> lo
    (mixed-dtype in, fp8 out).
  - psum[t, o] accumulates 32 DoubleRow matmuls (2 passes x 2 h x 8 b).
  - bias is pre-broadcast via a rank-1 f32r matmul into bbc; rowsum
    s[t] = sum_i bf16(x)[t, i] on Pool (consistent with hi+lo);
    epilogue fuses y = psum - s + bbc on DVE/Pool.
"""
import sys

sys.path.insert(0, "/opt/trn_rl_repo")
from contextlib import ExitStack

import numpy as np

import concourse.tile as tile
from concourse import bacc, mybir
from concourse.bass import ts
from concourse.bass_utils import run_bass_kernel_spmd
from concourse.masks import make_identity

F32 = mybir.dt.float32
F32R = mybir.dt.float32r
BF16 = mybir.dt.bfloat16
U16 = mybir.dt.uint16
F8 = mybir.dt.float8e4
P = 128

N_CORES = 8
B_DIM, S_DIM, I_DIM, O_DIM = 2, 2048, 4096, 4096
T_FULL = B_DIM * S_DIM          # 4096 tokens
T_SHARD = T_FULL // N_CORES     # 512 tokens per core
OUT_NAME = "y"


def build(T=T_SHARD, I=I_DIM, O=O_DIM, n_cores=N_CORES):
    H = I // 2048               # 2048-wide i-halves (j spans 16*128)
    TT = T // P                 # token tiles
    NB = 8                      # bits per byte
    K2 = I // 16                # u16 byte-pairs per weight row
    OSL = 512                   # o-slab width
    NSL = O // OSL

    nc = bacc.Bacc("TRN2", target_bir_lowering=False, debug=False,
                   num_devices=n_cores)
    x_d = nc.dram_tensor("x", [T, I], F32, kind="ExternalInput").ap()
    pw16_d = nc.dram_tensor("pw16", [O, K2], U16, kind="ExternalInput").ap()
    bias_d = nc.dram_tensor("bias", [O], F32, kind="ExternalInput").ap()
    y_d = nc.dram_tensor(OUT_NAME, [T, O], F32, kind="ExternalOutput").ap()

    with tile.TileContext(nc) as tc:
        with ExitStack() as ctx:
            const = ctx.enter_context(tc.tile_pool(name="const", bufs=1))
            persist = ctx.enter_context(tc.tile_pool(name="persist", bufs=1))
            stage = ctx.enter_context(tc.tile_pool(name="stage", bufs=1))

            ident_bf = const.tile([P, P], BF16)
            make_identity(nc, ident_bf[:])
            ones_r = const.tile([1, P], F32R)
            bias_r = const.tile([1, O], F32R)

            byteT2 = persist.tile([P, H, O], U16)
            # xT planes: [j, h, bh, tt, (b' p t)] fp8, 16KB/partition each
            xT_hi = persist.tile([P, H, 2, TT, 1024], F8)
            xT_lo = persist.tile([P, H, 2, TT, 1024], F8)

            xn_pool = ctx.enter_context(tc.tile_pool(name="xn", bufs=H * TT))
            wt_pool = ctx.enter_context(tc.tile_pool(name="wt", bufs=2))
            scol_pool = ctx.enter_context(tc.tile_pool(name="scol", bufs=1))
            trash_pool = ctx.enter_context(tc.tile_pool(name="trash", bufs=2))
            bbc_pool = ctx.enter_context(tc.tile_pool(name="bbc", bufs=2))
            y_pool = ctx.enter_context(tc.tile_pool(name="ysb", bufs=4))
            ps_tr = ctx.enter_context(
                tc.tile_pool(name="ps_tr", bufs=3, space="PSUM"))
            ps_mm = ctx.enter_context(
                tc.tile_pool(name="ps_mm", bufs=4, space="PSUM"))
            ps_b_pool = ctx.enter_context(
                tc.tile_pool(name="ps_b", bufs=1, space="PSUM"))

            # --- weight bytes: XBAR transpose-DMAs, slab-major ---
            for sl in range(NSL):
                for h in range(H):
                    nc.sync.dma_start_transpose(
                        byteT2[:, h, ts(sl, OSL)],
                        pw16_d[ts(sl, OSL), ts(h, P)],
                    )

            # --- x chunks: SWDGE f32->bf16 cast DMAs, h-major ---
            xns = {}
            for h in range(H):
                for tt in range(TT):
                    xn = xn_pool.tile([P, P, 16], BF16, tag="xn")
                    nc.gpsimd.dma_start(
                        xn[:].rearrange("t j q -> t (j q)"),
                        x_d[ts(tt, P), ts(h, 2048)],
                    )
                    xns[h, tt] = xn

            # --- bias/ones staging ---
            ones_f32 = stage.tile([1, P], F32)
            nc.vector.memset(ones_f32[:], 1.0)
            nc.vector.tensor_copy(out=ones_r[:], in_=ones_f32[:])
            bias_f32 = stage.tile([1, O], F32)
            nc.sync.dma_start(
                bias_f32[:], bias_d.rearrange("(b o) -> b o", b=1))
            nc.vector.tensor_copy(out=bias_r[:], in_=bias_f32[:])

            # --- unpack weight slabs 0,1 up front (DVE, 4x mode) ---
            def unpack(sl, wt):
                for h in range(H):
                    for b in range(NB):
                        if b < 7:
                            nc.vector.tensor_scalar(
                                out=wt[:, h, b, :],
                                in0=byteT2[:, h, ts(sl, OSL)],
                                scalar1=6 - b, scalar2=0x4040,
                                op0=mybir.AluOpType.logical_shift_left,
                                op1=mybir.AluOpType.bitwise_and,
                            )
                        else:
                            nc.vector.tensor_scalar(
                                out=wt[:, h, b, :],
                                in0=byteT2[:, h, ts(sl, OSL)],
                                scalar1=1, scalar2=0x4040,
                                op0=mybir.AluOpType.logical_shift_right,
                                op1=mybir.AluOpType.bitwise_and,
                            )

            wts = {}
            for sl in range(min(2, NSL)):
                wts[sl] = wt_pool.tile([P, H, NB, OSL], U16, name="wt", tag="wt")
                unpack(sl, wts[sl])

            # --- transposes + hi/lo split + rowsum, chunk-arrival order ---
            parts = scol_pool.tile([P, TT, H], F32)
            s_col = scol_pool.tile([P, TT], F32)
            for h in range(H):
                for tt in range(TT):
                    for bh in range(2):
                        ps = ps_tr.tile([P, 1024], BF16, tag="tr")
                        for bp in range(4):
                            for p in range(2):
                                nc.tensor.transpose(
                                    ps[:, ts(bp * 2 + p, P)],
                                    xns[h, tt][:, :, 8 * p + 4 * bh + bp],
                                    ident_bf[:],
                                )
                        nc.scalar.copy(out=xT_hi[:, h, bh, tt, :], in_=ps[:])
                        nc.vector.tensor_tensor(
                            out=xT_lo[:, h, bh, tt, :], in0=ps[:],
                            in1=xT_hi[:, h, bh, tt, :],
                            op=mybir.AluOpType.subtract,
                        )
                    # rowsum partial via tensor_scalar accumulator (2x mode)
                    trash = trash_pool.tile([P, P, 16], BF16, tag="trash")
                    nc.vector.tensor_scalar(
                        out=trash[:], in0=xns[h, tt][:],
                        scalar1=1.0, scalar2=0.0,
                        op0=mybir.AluOpType.mult, op1=mybir.AluOpType.add,
                        accum_out=parts[:, tt, h:h + 1],
                    )
                    if h == H - 1:
                        nc.vector.tensor_tensor(
                            out=s_col[:, tt:tt + 1], in0=parts[:, tt, 0:1],
                            in1=parts[:, tt, 1:2], op=mybir.AluOpType.add,
                        )

            # --- main loop over o-slabs ---
            for sl in range(NSL):
                wt = wts.pop(sl)
                ps_bias = ps_b_pool.tile([P, OSL], F32)
                nc.tensor.matmul(
                    ps_bias[:], ones_r[:], bias_r[:, ts(sl, OSL)],
                    start=True, stop=True,
                )
                bbc = bbc_pool.tile([P, OSL], F32, tag="bbc")
                nc.scalar.copy(out=bbc[:], in_=ps_bias[:])

                for tt in range(TT):
                    ps = ps_mm.tile([P, OSL], F32)
                    n = 0
                    for plane in (xT_hi, xT_lo):
                        for h in range(H):
                            for bh in range(2):
                                for bp in range(4):
                                    rhs = wt[:, h, 4 * bh + bp, :].bitcast(
                                        F8).rearrange("j (o p) -> j p o", p=2)
                                    nc.tensor.matmul(
                                        ps[:],
                                        plane[:, h, bh, tt, ts(bp, 256)]
                                        .rearrange("j (p t) -> j p t", p=2),
                                        rhs,
                                        start=(n == 0), stop=(n == 31),
                                        perf_mode=mybir.MatmulPerfMode.DoubleRow,
                                    )
                                    n += 1
                    y_sb = y_pool.tile([P, OSL], F32)
                    nc.vector.scalar_tensor_tensor(
                        out=y_sb[:], in0=ps[:], scalar=s_col[:, tt:tt + 1], in1=bbc[:],
                        op0=mybir.AluOpType.subtract,
                        op1=mybir.AluOpType.add,
                    )
                    nc.sync.dma_start(y_d[ts(tt, P), ts(sl, OSL)], y_sb[:])

                if sl + 2 < NSL:
                    wts[sl + 2] = wt_pool.tile([P, H, NB, OSL], U16, name="wt", tag="wt")
                    unpack(sl + 2, wts[sl + 2])

    nc.compile()
    return nc


_NC = None


def _get_nc():
    global _NC
    if _NC is None:
        _NC = build()
    return _NC


def run(x, packed_weight, bias, trace=False):
    x = np.ascontiguousarray(np.asarray(x, dtype=np.float32))
    pw = np.ascontiguousarray(np.asarray(packed_weight).astype(np.uint8))
    bias = np.ascontiguousarray(np.asarray(bias, dtype=np.float32))
    assert x.shape == (B_DIM, S_DIM, I_DIM)
    assert pw.shape == (O_DIM, I_DIM // 8)
    assert bias.shape == (O_DIM,)

    nc = _get_nc()
    xs = x.reshape(T_FULL, I_DIM)
    pw16 = pw.view(np.uint16)
    in_maps = [
        {
            "x": np.ascontiguousarray(xs[c * T_SHARD:(c + 1) * T_SHARD]),
            "pw16": pw16,
            "bias": bias,
        }
        for c in range(N_CORES)
    ]
    res = run_bass_kernel_spmd(nc, in_maps, list(range(N_CORES)), trace=trace)
    y = np.concatenate(
        [res.results[c][OUT_NAME] for c in range(N_CORES)], axis=0
    )
    return y.reshape(B_DIM, S_DIM, O_DIM), res


def kernel(x, packed_weight, bias):
    y, _ = run(x, packed_weight, bias, trace=False)
    return y


# revision 10
# speedup vs baseline: 1.8459x; 1.1262x over previous
"""BitPackedLinear Trainium2 kernel (8-core SPMD, token-sharded, fp8 DoubleRow).

y = x @ W.T + bias, W = unpack_bits(packed_weight) in {-1,+1}, shapes:
  x [2, 2048, 4096] f32, packed_weight [4096, 512] u8, bias [4096] f32.

Sharding: data-parallel over tokens (4096 tokens -> 512/core). Each core
computes y_c = x_c @ W.T + bias for its token shard against the full
weight; the host just concatenates shards.

Device algorithm per core (matmuls are fp8e4 DoubleRow at 0.5 cyc/row,
one instruction contracts TWO 128-deep k-tiles):
  - x is split as x_bf = hi + lo with hi = e4m3(x_bf), lo = x_bf - hi
    (lo is exactly representable in e4m3, so hi+lo == bf16(x) exactly;
    end-to-end rel err ~1.7e-3, dominated by the bf16 load cast).
  - i-tiling: i = 2048h + 16j + 8p + b with j the PE partition, (p, b)
    byte-parity/bit, h the 2048-halves. The DoubleRow k-tile pair is p.
  - weights: pw is viewed as u16 [4096, 256] (byte pairs along k) and
    XBAR-transpose-DMA'd to byteT2 [128j, h, o]; each u16 holds bytes
    (2j, 2j+1) of one o. A single u16 shift+mask (<<(6-b), & 0x4040)
    then yields the e4m3 BIT PATTERN of {0, 2.0} (0x40) for BOTH
    parities of bit b at once -> one DVE op per (h, b, o-slab), all in
    the DVE 4x perf mode. The u16 result bitcast to fp8 is the moving
    [j, p, o] operand.
  - x chunks arrive as f32->bf16 SWDGE cast-DMAs [128t, 2048i]; PE
    transposes (1 cyc/row) produce [j, t] bf16 tiles grouped in wide
    PSUM tiles; ACT casts psum->fp8 (hi), DVE subtracts psum - hi -> lo
    (mixed-dtype in, fp8 out).
  - psum[t, o] accumulates 32 DoubleRow matmuls (2 passes x 2 h x 8 b).
  - bias is pre-broadcast via a rank-1 f32r matmul into bbc; rowsum
    s[t] = sum_i bf16(x)[t, i] on Pool (consistent with hi+lo);
    epilogue fuses y = psum - s + bbc on DVE/Pool.
"""
import sys

sys.path.insert(0, "/opt/trn_rl_repo")
from contextlib import ExitStack

import numpy as np

import concourse.tile as tile
from concourse import bacc, mybir
from concourse.bass import ts
from concourse.bass_utils import run_bass_kernel_spmd
from concourse.masks import make_identity

F32 = mybir.dt.float32
F32R = mybir.dt.float32r
BF16 = mybir.dt.bfloat16
U16 = mybir.dt.uint16
F8 = mybir.dt.float8e4
P = 128

N_CORES = 8
B_DIM, S_DIM, I_DIM, O_DIM = 2, 2048, 4096, 4096
T_FULL = B_DIM * S_DIM          # 4096 tokens
T_SHARD = T_FULL // N_CORES     # 512 tokens per core
OUT_NAME = "y"


def build(T=T_SHARD, I=I_DIM, O=O_DIM, n_cores=N_CORES):
    H = I // 2048               # 2048-wide i-halves (j spans 16*128)
    TT = T // P                 # token tiles
    NB = 8                      # bits per byte
    K2 = I // 16                # u16 byte-pairs per weight row
    OSL = 512                   # o-slab width
    NSL = O // OSL

    nc = bacc.Bacc("TRN2", target_bir_lowering=False, debug=False,
                   num_devices=n_cores)
    x_d = nc.dram_tensor("x", [T, I], F32, kind="ExternalInput").ap()
    pw16_d = nc.dram_tensor("pw16", [O, K2], U16, kind="ExternalInput").ap()
    bias_d = nc.dram_tensor("bias", [O], F32, kind="ExternalInput").ap()
    y_d = nc.dram_tensor(OUT_NAME, [T, O], F32, kind="ExternalOutput").ap()

    with tile.TileContext(nc) as tc:
        with ExitStack() as ctx:
            const = ctx.enter_context(tc.tile_pool(name="const", bufs=1))
            persist = ctx.enter_context(tc.tile_pool(name="persist", bufs=1))
            stage = ctx.enter_context(tc.tile_pool(name="stage", bufs=1))

            ident_bf = const.tile([P, P], BF16)
            make_identity(nc, ident_bf[:])
            ones_r = const.tile([1, P], F32R)
            bias_r = const.tile([1, O], F32R)

            byteT2 = persist.tile([P, H, O], U16)
            # xT planes: [j, h, bh, tt, (b' p t)] fp8, 16KB/partition each
            xT_hi = persist.tile([P, H, 2, TT, 1024], F8)
            xT_lo = persist.tile([P, H, 2, TT, 1024], F8)

            xn_pool = ctx.enter_context(tc.tile_pool(name="xn", bufs=H * TT))
            wt_pool = ctx.enter_context(tc.tile_pool(name="wt", bufs=2))
            scol_pool = ctx.enter_context(tc.tile_pool(name="scol", bufs=1))
            trash_pool = ctx.enter_context(tc.tile_pool(name="trash", bufs=2))
            bbc_pool = ctx.enter_context(tc.tile_pool(name="bbc", bufs=2))
            y_pool = ctx.enter_context(tc.tile_pool(name="ysb", bufs=4))
            ps_tr = ctx.enter_context(
                tc.tile_pool(name="ps_tr", bufs=3, space="PSUM"))
            ps_mm = ctx.enter_context(
                tc.tile_pool(name="ps_mm", bufs=4, space="PSUM"))
            ps_b_pool = ctx.enter_context(
                tc.tile_pool(name="ps_b", bufs=1, space="PSUM"))

            # --- weight bytes: one XBAR transpose-DMA per i-half ---
            for h in range(H):
                nc.sync.dma_start_transpose(
                    byteT2[:, h, :], pw16_d[:, ts(h, P)])

            # --- x chunks: SWDGE f32->bf16 cast DMAs, h-major ---
            xns = {}
            for h in range(H):
                for tt in range(TT):
                    xn = xn_pool.tile([P, P, 16], BF16, tag="xn")
                    nc.gpsimd.dma_start(
                        xn[:].rearrange("t j q -> t (j q)"),
                        x_d[ts(tt, P), ts(h, 2048)],
                    )
                    xns[h, tt] = xn

            # --- bias/ones staging ---
            ones_f32 = stage.tile([1, P], F32)
            nc.vector.memset(ones_f32[:], 1.0)
            nc.vector.tensor_copy(out=ones_r[:], in_=ones_f32[:])
            bias_f32 = stage.tile([1, O], F32)
            nc.sync.dma_start(
                bias_f32[:], bias_d.rearrange("(b o) -> b o", b=1))
            nc.vector.tensor_copy(out=bias_r[:], in_=bias_f32[:])

            # --- unpack weight slabs 0,1 up front (DVE, 4x mode) ---
            def unpack(sl, wt):
                for h in range(H):
                    for b in range(NB):
                        if b < 7:
                            nc.vector.tensor_scalar(
                                out=wt[:, h, b, :],
                                in0=byteT2[:, h, ts(sl, OSL)],
                                scalar1=6 - b, scalar2=0x4040,
                                op0=mybir.AluOpType.logical_shift_left,
                                op1=mybir.AluOpType.bitwise_and,
                            )
                        else:
                            nc.vector.tensor_scalar(
                                out=wt[:, h, b, :],
                                in0=byteT2[:, h, ts(sl, OSL)],
                                scalar1=1, scalar2=0x4040,
                                op0=mybir.AluOpType.logical_shift_right,
                                op1=mybir.AluOpType.bitwise_and,
                            )

            wts = {}
            for sl in range(min(2, NSL)):
                wts[sl] = wt_pool.tile([P, H, NB, OSL], U16, name="wt", tag="wt")
                unpack(sl, wts[sl])

            # --- transposes + hi/lo split + rowsum, chunk-arrival order ---
            parts = scol_pool.tile([P, TT, H], F32)
            s_col = scol_pool.tile([P, TT], F32)
            for h in range(H):
                for tt in range(TT):
                    for bh in range(2):
                        ps = ps_tr.tile([P, 1024], BF16, tag="tr")
                        for bp in range(4):
                            for p in range(2):
                                nc.tensor.transpose(
                                    ps[:, ts(bp * 2 + p, P)],
                                    xns[h, tt][:, :, 8 * p + 4 * bh + bp],
                                    ident_bf[:],
                                )
                        nc.scalar.copy(out=xT_hi[:, h, bh, tt, :], in_=ps[:])
                        nc.vector.tensor_tensor(
                            out=xT_lo[:, h, bh, tt, :], in0=ps[:],
                            in1=xT_hi[:, h, bh, tt, :],
                            op=mybir.AluOpType.subtract,
                        )
                    # rowsum partial via tensor_scalar accumulator (2x mode)
                    trash = trash_pool.tile([P, P, 16], BF16, tag="trash")
                    nc.vector.tensor_scalar(
                        out=trash[:], in0=xns[h, tt][:],
                        scalar1=1.0, scalar2=0.0,
                        op0=mybir.AluOpType.mult, op1=mybir.AluOpType.add,
                        accum_out=parts[:, tt, h:h + 1],
                    )
                    if h == H - 1:
                        nc.vector.tensor_tensor(
                            out=s_col[:, tt:tt + 1], in0=parts[:, tt, 0:1],
                            in1=parts[:, tt, 1:2], op=mybir.AluOpType.add,
                        )

            # --- main loop over o-slabs ---
            for sl in range(NSL):
                wt = wts.pop(sl)
                ps_bias = ps_b_pool.tile([P, OSL], F32)
                nc.tensor.matmul(
                    ps_bias[:], ones_r[:], bias_r[:, ts(sl, OSL)],
                    start=True, stop=True,
                )
                bbc = bbc_pool.tile([P, OSL], F32, tag="bbc")
                nc.scalar.copy(out=bbc[:], in_=ps_bias[:])

                for tt in range(TT):
                    ps = ps_mm.tile([P, OSL], F32)
                    n = 0
                    for plane in (xT_hi, xT_lo):
                        for h in range(H):
                            for bh in range(2):
                                for bp in range(4):
                                    rhs = wt[:, h, 4 * bh + bp, :].bitcast(
                                        F8).rearrange("j (o p) -> j p o", p=2)
                                    nc.tensor.matmul(
                                        ps[:],
                                        plane[:, h, bh, tt, ts(bp, 256)]
                                        .rearrange("j (p t) -> j p t", p=2),
                                        rhs,
                                        start=(n == 0), stop=(n == 31),
                                        perf_mode=mybir.MatmulPerfMode.DoubleRow,
                                    )
                                    n += 1
                    y_sb = y_pool.tile([P, OSL], F32)
                    nc.vector.scalar_tensor_tensor(
                        out=y_sb[:], in0=ps[:], scalar=s_col[:, tt:tt + 1], in1=bbc[:],
                        op0=mybir.AluOpType.subtract,
                        op1=mybir.AluOpType.add,
                    )
                    nc.sync.dma_start(y_d[ts(tt, P), ts(sl, OSL)], y_sb[:])

                if sl + 2 < NSL:
                    wts[sl + 2] = wt_pool.tile([P, H, NB, OSL], U16, name="wt", tag="wt")
                    unpack(sl + 2, wts[sl + 2])

    nc.compile()
    return nc


_NC = None


def _get_nc():
    global _NC
    if _NC is None:
        _NC = build()
    return _NC


def run(x, packed_weight, bias, trace=False):
    x = np.ascontiguousarray(np.asarray(x, dtype=np.float32))
    pw = np.ascontiguousarray(np.asarray(packed_weight).astype(np.uint8))
    bias = np.ascontiguousarray(np.asarray(bias, dtype=np.float32))
    assert x.shape == (B_DIM, S_DIM, I_DIM)
    assert pw.shape == (O_DIM, I_DIM // 8)
    assert bias.shape == (O_DIM,)

    nc = _get_nc()
    xs = x.reshape(T_FULL, I_DIM)
    pw16 = pw.view(np.uint16)
    in_maps = [
        {
            "x": np.ascontiguousarray(xs[c * T_SHARD:(c + 1) * T_SHARD]),
            "pw16": pw16,
            "bias": bias,
        }
        for c in range(N_CORES)
    ]
    res = run_bass_kernel_spmd(nc, in_maps, list(range(N_CORES)), trace=trace)
    y = np.concatenate(
        [res.results[c][OUT_NAME] for c in range(N_CORES)], axis=0
    )
    return y.reshape(B_DIM, S_DIM, O_DIM), res


def kernel(x, packed_weight, bias):
    y, _ = run(x, packed_weight, bias, trace=False)
    return y
